# revision 3
# baseline (speedup 1.0000x reference)
"""Causal single-head attention (B=8, T=2048, C=1024, HS=64) on 8 trn2 cores.

v4: fp16 streaming pipeline, data-parallel over batch B (one element/core).

Per core, per 512-row group g, for x in (k, v, q):
  1. cast-load x group [t, C] fp32 -> fp16 SBUF (SWDGE cast DMA on Pool)
  2. PE is_transpose (fp16, exact) against an fp16 identity, PSUM evacuated
     [128, 1024] at a time on DVE (2x 16-bit mode) / ACT -> xT [c-part, cb, t]
  3. projections on PE (fp16 operands, fp32 PSUM accum over 8 c-chunks)
     -> qt/kt fp16 [64, 512]; V is PE-transposed back to natural [j, h] and
     extended with a mask/ones column so the softmax denominator falls out
     of the A@V matmul
  4. attention i-chunk ic=g: scores^T per j-block (fp16), one PAIRED exp on
     ACT per two j-blocks (fused 1/8 scale, fp16 out), causal diagonal via a
     combined [umask|ones ; 0|umask] multiplicative mask, A@V accumulated
     over j-blocks in fp32 PSUM
  5. out^T transposed back on PE, divided by the denominator column, stored
     via SP HWDGE

Group g's attention needs only k/v groups <= g and q group g, so loads,
transposes, projections and attention all stream; next-group frontend units
are drained into the PE's exp-wait gaps during attention emission.
"""

import numpy as np

import concourse.bass as bass
import concourse.mybir as mybir
import concourse.tile as tile

B, T, C, HS = 8, 2048, 1024, 64
P = 128
NT = T // P  # 16 t-tiles
NCB = C // P  # 8 c-chunks
TI = 512  # i-chunk width
GG = 4  # t-tiles per group
NG = NT // GG  # 4 groups

# scheduling knobs (tuned via TimelineSim sweep)
EVAC_MOD = 3  # 1 of EVAC_MOD transpose evacs goes to ACT (0 = all DVE)
DRAIN_N = 6  # frontend units drained per attention pair
XT_BUFS = 4
EX_BUFS = 4
MUL_SPLIT = 1  # final muls: every MUL_SPLIT-th on DVE, rest ACT

F32 = mybir.dt.float32
FP16 = mybir.dt.bfloat16  # bf16: fp16-cast DMA is slow on HW
I32 = mybir.dt.int32
EXP = mybir.ActivationFunctionType.Exp
COPY = mybir.ActivationFunctionType.Copy


def split_excess_waits(nc):
    """walrus supports 1 sync-wait per instruction (2 on EventSemaphore);
    Tile's final drain can accumulate more. Hoist excess waits onto NoOp
    carriers inserted immediately before the overloaded instruction."""
    for blk in nc.m.functions[0].blocks:
        insts = blk.instructions
        i = 0
        while i < len(insts):
            inst = insts[i]
            si = inst.sync_info
            cap = 2 if isinstance(inst, mybir.InstEventSemaphore) else 1
            if si is not None and si.on_wait and len(si.on_wait) > cap:
                waits = list(si.on_wait)
                si.on_wait = waits[:cap]
                carriers = []
                for w in waits[cap:]:
                    n = mybir.InstNoOp(
                        name=nc.get_next_instruction_name(), ins=[], outs=[]
                    )
                    n.engine = inst.engine
                    n.sync_info = mybir.SyncInfo(on_wait=[w], on_update=[])
                    nc.register_instruction(n)
                    carriers.append(n)
                for j, n in enumerate(carriers):
                    insts.insert(i + j, n)
                i += len(carriers)
            i += 1


def make_tri(nc, out, kind):
    """identity / upper-triangular constant, built in-place on Pool."""
    sq = out.shape[0]
    nc.gpsimd.memset(out, 0.0)
    nc.gpsimd.affine_select(
        out=out,
        in_=out,
        compare_op=(
            mybir.AluOpType.not_equal if kind == "ident" else mybir.AluOpType.is_gt
        ),
        fill=1.0,
        base=0,
        pattern=[[-1, sq]],
        channel_multiplier=1,
    )


def attention_body(tc, consts, pools, q, k, v, out, phase=4):
    """Emit one iteration (per-core shapes). Pools persist across iterations
    so back-to-back iterations pipeline."""
    nc = tc.nc
    identh = consts["identh"]
    umask = consts["umask"]
    mask2 = consts["mask2"]
    w_list = consts["w_list"]
    mask_f = consts["mask_f"]

    (nat_pool, xt_pool, pj_pool, vx_pool, ex_pool, on_pool,
     ps_tr, ps_pj, ps_sc, ps_out) = pools

    # --- all input loads up front (Pool SWDGE cast fp32->fp16) -------------
    srcs = {"k": k, "v": v, "q": q}
    natg = {}
    for g in range(NG):
        for name in ("k", "v", "q"):
            x = srcs[name]
            ng = nat_pool.tile([P, GG, C], FP16, tag="natg", bufs=3 * NG, name="ng")
            nc.gpsimd.dma_start(
                out=ng[:],
                in_=x[g * GG * P : (g + 1) * GG * P, :].rearrange(
                    "(tt p) c -> p tt c", p=P
                ),
            )
            natg[(name, g)] = ng
            if g > 0:
                continue
            widx = {"q": 0, "k": 1, "v": 2}[name]
            if w_list[widx][1] is None:
                w_dram = w_list[widx][0]
                # [c-part, cb, h] fp32 on SP HWDGE (no Pool time), ACT -> fp16
                wf = nat_pool.tile(
                    [P, NCB, HS], F32, tag=f"wf_{name}", bufs=1, name="wf"
                )
                nc.sync.dma_start(
                    out=wf[:], in_=w_dram.rearrange("(cb c) h -> c cb h", c=P)
                )
                wh = nat_pool.tile(
                    [P, NCB, HS], FP16, tag=f"wh_{name}", bufs=1, name="wh"
                )
                nc.scalar.copy(out=wh[:], in_=wf[:])
                w_list[widx] = (w_dram, wh)
            if name == "k" and not consts["built_a"]:
                # identity right behind the first load's descriptor gen
                consts["built_a"] = True
                make_tri(nc, identh[:], "ident")
            if name == "q" and not consts["built_b"]:
                # remaining consts behind the q-load's descriptor gen
                consts["built_b"] = True
                make_tri(nc, umask[:], "umask")
                nc.gpsimd.memset(mask2[:], 1.0)
                nc.gpsimd.memset(mask2[:, 1, 0:P], 0.0)
                nc.vector.tensor_copy(out=mask2[:, 0, 0:P], in_=umask[:])
                nc.vector.tensor_copy(out=mask2[:, 1, P : 2 * P], in_=umask[:])
                mask_i = consts["mask_i"]
                nc.sync.dma_start(
                    out=mask_i[:],
                    in_=consts["mask_dram"].rearrange("(tb p) -> p tb", p=P),
                )
                nc.vector.tensor_copy(out=mask_f[:], in_=mask_i[:])

    if phase < 2:
        dummy = on_pool.tile([P, HS], F32, tag="osb_d")
        nc.vector.tensor_copy(out=dummy[:], in_=natg[("q", NG - 1)][:, 0, 0:HS])
        nc.sync.dma_start(out=out[0:P, :], in_=dummy[:])
        return

    qts, kts, vxs = [], [], []
    evac = [0]

    def frontend_units(g):
        """Emission closures for group g's transposes + projections, so
        attention(g-1) can interleave them into the PE's exp-wait gaps."""
        units = []

        def mk_transpose(name, cbp, holder):
            def u():
                if cbp == 0:
                    holder["xt"] = xt_pool.tile(
                        [P, NCB, TI], FP16, tag="xt", name="xt"
                    )
                xt = holder["xt"]
                tp = ps_tr.tile([P, 2, GG, P], FP16, tag="tr")
                for ci in range(2):
                    cb = 2 * cbp + ci
                    for tt in range(GG):
                        nc.tensor.transpose(
                            tp[:, ci, tt, :],
                            natg[(name, g)][:, tt, cb * P : (cb + 1) * P],
                            identh[:],
                        )
                dst = xt[:, 2 * cbp : 2 * cbp + 2, :].rearrange(
                    "p c (tt ti) -> p c tt ti", tt=GG
                )
                if EVAC_MOD and evac[0] % EVAC_MOD == EVAC_MOD - 1:
                    nc.scalar.copy(out=dst, in_=tp[:])
                else:
                    nc.vector.tensor_copy(out=dst, in_=tp[:])
                evac[0] += 1
            return u

        def mk_proj(name, holder):
            def u():
                wh = w_list[{"q": 0, "k": 1, "v": 2}[name]][1]
                xt = holder["xt"]
                pj = ps_pj.tile([HS, TI], F32, tag="pj")
                for cb in range(NCB):
                    nc.tensor.matmul(
                        pj[:],
                        lhsT=wh[:, cb, :],
                        rhs=xt[:, cb, :],
                        start=(cb == 0),
                        stop=(cb == NCB - 1),
                    )
                if name in ("q", "k"):
                    pt = pj_pool.tile([HS, TI], FP16, tag=f"{name}t", bufs=2 * NG)
                    nc.vector.tensor_copy(out=pt[:], in_=pj[:])
                    (qts if name == "q" else kts).append(pt)
                else:
                    pass
            return u

        def mk_vproj(holder):
            def u():
                # V projected directly into natural [j-part, h]: the xT
                # chunks are the stationary operand and W streams, so no
                # transpose-back is needed
                wh = w_list[2][1]
                xt = holder["xt"]
                vxg = vx_pool.tile([P, GG, HS + 4], FP16, tag="vx", bufs=2 * NG)
                vp = ps_pj.tile([P, GG, HS + 2], F32, tag="pj", name="vp")
                for tt in range(GG):
                    for cb in range(NCB):
                        nc.tensor.matmul(
                            vp[:, tt, 0:HS],
                            lhsT=xt[:, cb, tt * P : (tt + 1) * P],
                            rhs=wh[:, cb, :],
                            start=(cb == 0),
                            stop=(cb == NCB - 1),
                        )
                for tt in range(GG):
                    jt = g * GG + tt
                    nc.vector.tensor_scalar_mul(
                        out=vxg[:, tt, 0:HS],
                        in0=vp[:, tt, 0:HS],
                        scalar1=mask_f[:, jt : jt + 1],
                    )
                nc.vector.tensor_copy(
                    out=vxg[:, :, HS], in_=mask_f[:, g * GG : (g + 1) * GG]
                )
                vxs.append(vxg)
            return u

        for name in ("k", "v", "q"):
            holder = {}
            for cbp in range(NCB // 2):
                units.append(mk_transpose(name, cbp, holder))
            if phase >= 3:
                units.append(
                    mk_vproj(holder) if name == "v" else mk_proj(name, holder)
                )
        return units

    def attention(g, units):
        """Attention i-chunk ic == g: paired fp16 scores/exp one pair ahead
        of A@V, with next-group frontend units drained into exp-wait gaps."""
        njb = 4 * g + 4
        out_ps = ps_out.tile([HS + 1, TI], F32, tag="out")
        exs = {}

        def drain(n):
            for _ in range(n):
                if units:
                    units.pop(0)()

        def pair_scores(pr):
            """Both j-blocks of pair pr -> fp32 PSUM, one paired exp -> fp16.
            Diagonal pairs compute the full [o0:) range for both sub-blocks
            and apply one combined [umask|ones ; 0|umask] multiplicative
            mask over the 256 columns at o0."""
            jb0 = 2 * pr
            o0 = max(0, jb0 * P - g * TI)
            sc = ps_sc.tile([P, 2, TI], F32, tag="sc")
            for i in (0, 1):
                gj, tj = (jb0 + i) // 4, (jb0 + i) % 4
                nc.tensor.matmul(
                    sc[:, i, o0:],
                    lhsT=kts[gj][:, tj * P : (tj + 1) * P],
                    rhs=qts[g][:, o0:],
                    start=True,
                    stop=True,
                )
            ex = ex_pool.tile([P, 2, TI], FP16, tag="ex")
            nc.scalar.activation(
                out=ex[:, :, o0:],
                in_=sc[:, :, o0:],
                func=EXP,
                scale=float(HS) ** -0.5,
            )
            if jb0 >= 4 * g:
                nc.vector.tensor_mul(
                    ex[:, :, o0 : o0 + 2 * P], ex[:, :, o0 : o0 + 2 * P], mask2[:]
                )
            exs[pr] = (ex, o0)

        def av(pr, npr):
            ex, o0 = exs.pop(pr)
            for i in (0, 1):
                jb = 2 * pr + i
                o = max(0, jb * P - g * TI)
                gj, tj = jb // 4, jb % 4
                nc.tensor.matmul(
                    out_ps[:, o:],
                    lhsT=vxs[gj][:, tj, 0 : HS + 1],
                    rhs=ex[:, i, o:],
                    start=(jb == 0),
                    stop=(jb == njb - 1),
                )

        npr = njb // 2
        pair_scores(0)
        for pr in range(1, npr):
            pair_scores(pr)
            drain(DRAIN_N)
            av(pr - 1, npr)
        av(npr - 1, npr)

        # normalize + emit
        oun = on_pool.tile([HS + 1, TI], FP16, tag="oun")
        nc.scalar.copy(out=oun[:], in_=out_ps[:])
        osb = on_pool.tile([P, GG, HS], F32, tag="osb")
        op = ps_out.tile([P, GG, HS + 2], FP16, tag="out", name="op")
        for tt in range(GG):
            nc.tensor.transpose(
                op[:, tt, 0 : HS + 1],
                oun[:, tt * P : (tt + 1) * P],
                identh[0 : HS + 1, 0 : HS + 1],
            )
        rden = on_pool.tile([P, GG], F32, tag="rden")
        nc.vector.reciprocal(out=rden[:], in_=op[:, :, HS])
        for tt in range(GG):
            if tt % MUL_SPLIT == 0:
                nc.vector.tensor_scalar_mul(
                    out=osb[:, tt, :], in0=op[:, tt, 0:HS],
                    scalar1=rden[:, tt : tt + 1],
                )
            else:
                nc.scalar.mul(
                    out=osb[:, tt, :], in_=op[:, tt, 0:HS],
                    mul=rden[:, tt : tt + 1],
                )
        nc.sync.dma_start(
            out=out[g * TI : (g + 1) * TI, :].rearrange("(tt p) h -> p tt h", p=P),
            in_=osb[:],
        )

    # --- pipeline: next-group frontend interleaved into attention ----------
    for u in frontend_units(0):
        u()
    for g in range(NG):
        units = frontend_units(g + 1) if g + 1 < NG else []
        if phase < 4:
            for u in units:
                u()
            if phase >= 3 and g == NG - 1:
                dummy = on_pool.tile([P, HS], F32, tag="osb_d")
                nc.vector.tensor_copy(out=dummy[:], in_=vxs[g][:, 0, 0:HS])
                nc.sync.dma_start(out=out[0:P, :], in_=dummy[:])
            continue
        attention(g, units)
        for u in units:
            u()


def build_nc(n_iters: int = 1, phase: int = 4):
    nc = bass.Bass(trn_type="TRN2", num_devices=B)
    q = nc.declare_dram_parameter("q_vec", [T, C], F32, isOutput=False)
    k = nc.declare_dram_parameter("k_vec", [T, C], F32, isOutput=False)
    v = nc.declare_dram_parameter("v_vec", [T, C], F32, isOutput=False)
    mask = nc.declare_dram_parameter("mask", [T], I32, isOutput=False)
    wq = nc.declare_dram_parameter("Wq", [C, HS], F32, isOutput=False)
    wk = nc.declare_dram_parameter("Wk", [C, HS], F32, isOutput=False)
    wv = nc.declare_dram_parameter("Wv", [C, HS], F32, isOutput=False)
    out = nc.declare_dram_parameter("out", [T, HS], F32, isOutput=True)

    with tile.TileContext(nc) as tc:
        with tc.tile_pool(name="singles", bufs=1) as singles, \
             tc.tile_pool(name="nat", bufs=1) as nat_pool, \
             tc.tile_pool(name="xt", bufs=XT_BUFS) as xt_pool, \
             tc.tile_pool(name="pj", bufs=2) as pj_pool, \
             tc.tile_pool(name="vx", bufs=2) as vx_pool, \
             tc.tile_pool(name="ex", bufs=EX_BUFS) as ex_pool, \
             tc.tile_pool(name="on", bufs=2) as on_pool, \
             tc.tile_pool(name="ps_tr", bufs=2, space="PSUM") as ps_tr, \
             tc.tile_pool(name="ps_pj", bufs=1, space="PSUM") as ps_pj, \
             tc.tile_pool(name="ps_sc", bufs=2, space="PSUM") as ps_sc, \
             tc.tile_pool(name="ps_out", bufs=1, space="PSUM") as ps_out:
            # all constants are emitted lazily inside the first body,
            # interleaved with the startup-critical group-0 loads
            consts = {
                "built_a": False,
                "built_b": False,
                "identh": singles.tile([P, P], FP16, name="identh"),
                "umask": singles.tile([P, P], FP16, name="umask"),
                "mask2": singles.tile([P, 2, 2 * P], FP16, name="mask2"),
                "mask_i": singles.tile([P, NT], I32, name="mask_i"),
                "mask_f": singles.tile([P, NT], F32, name="mask_f"),
                "mask_dram": mask.ap(),
                "w_list": [(wq.ap(), None), (wk.ap(), None), (wv.ap(), None)],
            }
            pools = (nat_pool, xt_pool, pj_pool, vx_pool, ex_pool, on_pool,
                     ps_tr, ps_pj, ps_sc, ps_out)
            for _ in range(n_iters):
                attention_body(
                    tc, consts, pools, q.ap(), k.ap(), v.ap(), out.ap(), phase=phase
                )

    split_excess_waits(nc)
    return nc


# ---------------------------------------------------------------------------
# SPMD runner (compile once, execute via PJRT on the 8 axon cores)
# ---------------------------------------------------------------------------
class _Runner:
    def __init__(self, nc, n_cores=B):
        import jax
        from jax.sharding import Mesh, PartitionSpec
        from jax.experimental.shard_map import shard_map
        from concourse.bass2jax import (
            _bass_exec_p,
            install_neuronx_cc_hook,
            partition_id_tensor,
        )

        install_neuronx_cc_hook()
        self.jax = jax
        self.n_cores = n_cores
        partition_name = (
            nc.partition_id_tensor.name if nc.partition_id_tensor else None
        )

        in_names, out_names, out_avals, zero_outs = [], [], [], []
        for alloc in nc.m.functions[0].allocations:
            if not isinstance(alloc, mybir.MemoryLocationSet):
                continue
            name = alloc.memorylocations[0].name
            if alloc.kind == "ExternalInput":
                if name != partition_name:
                    in_names.append(name)
            elif alloc.kind == "ExternalOutput":
                out_names.append(name)
                shape = tuple(alloc.tensor_shape)
                dtype = mybir.dt.np(alloc.dtype)
                out_avals.append(jax.core.ShapedArray(shape, dtype))
                zero_outs.append(np.zeros(shape, dtype))
        self.in_names = list(in_names)
        self.out_names = out_names
        self.out_avals = out_avals
        self.zero_outs = zero_outs
        n_params = len(in_names)
        self.n_params = n_params

        all_in_names = list(in_names) + list(out_names)
        if partition_name is not None:
            all_in_names.append(partition_name)

        def _body(*args):
            operands = list(args)
            if partition_name is not None:
                operands.append(partition_id_tensor())
            outs = _bass_exec_p.bind(
                *operands,
                out_avals=tuple(out_avals),
                in_names=tuple(all_in_names),
                out_names=tuple(out_names),
                lowering_input_output_aliases=(),
                sim_require_finite=True,
                sim_require_nnan=True,
                nc=nc,
            )
            return tuple(outs)

        devices = jax.devices()[:n_cores]
        mesh = Mesh(np.asarray(devices), ("core",))
        n_outs = len(out_names)
        self.fn = jax.jit(
            shard_map(
                _body,
                mesh=mesh,
                in_specs=(PartitionSpec("core"),) * (n_params + n_outs),
                out_specs=(PartitionSpec("core"),) * n_outs,
                check_rep=False,
            ),
            keep_unused=True,
        )

    def prepare(self, in_maps):
        n = self.n_cores
        per_core = [[np.asarray(m[nm]) for nm in self.in_names] for m in in_maps]
        concat_in = [
            np.concatenate([per_core[c][i] for c in range(n)], axis=0)
            for i in range(self.n_params)
        ]
        concat_zeros = [
            np.zeros((n * z.shape[0], *z.shape[1:]), z.dtype) for z in self.zero_outs
        ]
        self.args = [self.jax.device_put(a) for a in concat_in + concat_zeros]
        return self

    def run(self):
        outs = self.fn(*self.args)
        self.jax.block_until_ready(outs)
        return outs

    def results(self, outs):
        n = self.n_cores
        return [
            {
                nm: np.asarray(outs[i]).reshape(n, *self.out_avals[i].shape)[c]
                for i, nm in enumerate(self.out_names)
            }
            for c in range(n)
        ]


_CACHED = {}


def _get_runner(n_iters: int = 1, phase: int = 4):
    key = (n_iters, phase)
    if key not in _CACHED:
        _CACHED[key] = _Runner(build_nc(n_iters, phase))
    return _CACHED[key]


def kernel(q_vec, k_vec, v_vec, mask, Wq, Wk, Wv):
    q_vec = np.ascontiguousarray(np.asarray(q_vec, dtype=np.float32))
    k_vec = np.ascontiguousarray(np.asarray(k_vec, dtype=np.float32))
    v_vec = np.ascontiguousarray(np.asarray(v_vec, dtype=np.float32))
    mask = np.ascontiguousarray(np.asarray(mask, dtype=np.int32))
    Wq = np.ascontiguousarray(np.asarray(Wq, dtype=np.float32))
    Wk = np.ascontiguousarray(np.asarray(Wk, dtype=np.float32))
    Wv = np.ascontiguousarray(np.asarray(Wv, dtype=np.float32))

    r = _get_runner()
    in_maps = [
        {
            "q_vec": q_vec[b],
            "k_vec": k_vec[b],
            "v_vec": v_vec[b],
            "mask": mask[b],
            "Wq": Wq,
            "Wk": Wk,
            "Wv": Wv,
        }
        for b in range(B)
    ]
    r.prepare(in_maps)
    res = r.results(r.run())
    return np.stack([res[b]["out"] for b in range(B)], axis=0)


# revision 4
# speedup vs baseline: 2.1601x; 2.1601x over previous
"""Causal single-head attention (B=8, T=2048, C=1024, HS=64) on 8 trn2 cores.

v4: fp16 streaming pipeline, data-parallel over batch B (one element/core).

Per core, per 512-row group g, for x in (k, v, q):
  1. cast-load x group [t, C] fp32 -> fp16 SBUF (SWDGE cast DMA on Pool)
  2. PE is_transpose (fp16, exact) against an fp16 identity, PSUM evacuated
     [128, 1024] at a time on DVE (2x 16-bit mode) / ACT -> xT [c-part, cb, t]
  3. projections on PE (fp16 operands, fp32 PSUM accum over 8 c-chunks)
     -> qt/kt fp16 [64, 512]; V is PE-transposed back to natural [j, h] and
     extended with a mask/ones column so the softmax denominator falls out
     of the A@V matmul
  4. attention i-chunk ic=g: scores^T per j-block (fp16), one PAIRED exp on
     ACT per two j-blocks (fused 1/8 scale, fp16 out), causal diagonal via a
     combined [umask|ones ; 0|umask] multiplicative mask, A@V accumulated
     over j-blocks in fp32 PSUM
  5. out^T transposed back on PE, divided by the denominator column, stored
     via SP HWDGE

Group g's attention needs only k/v groups <= g and q group g, so loads,
transposes, projections and attention all stream; next-group frontend units
are drained into the PE's exp-wait gaps during attention emission.
"""

import numpy as np

import concourse.bass as bass
import concourse.mybir as mybir
import concourse.tile as tile

B, T, C, HS = 8, 2048, 1024, 64
P = 128
NT = T // P  # 16 t-tiles
NCB = C // P  # 8 c-chunks
TI = 512  # i-chunk width
GG = 4  # t-tiles per group
NG = NT // GG  # 4 groups

# scheduling knobs (tuned via TimelineSim sweep)
EVAC_MOD = 3  # 1 of EVAC_MOD transpose evacs goes to ACT (0 = all DVE)
DRAIN_N = 3  # frontend units drained per attention pair
XT_BUFS = 4
EX_BUFS = 4
MUL_SPLIT = 1  # final muls: every MUL_SPLIT-th on DVE, rest ACT

F32 = mybir.dt.float32
FP16 = mybir.dt.bfloat16  # bf16: fp16-cast DMA is slow on HW
I32 = mybir.dt.int32
EXP = mybir.ActivationFunctionType.Exp
COPY = mybir.ActivationFunctionType.Copy


def split_excess_waits(nc):
    """walrus supports 1 sync-wait per instruction (2 on EventSemaphore);
    Tile's final drain can accumulate more. Hoist excess waits onto NoOp
    carriers inserted immediately before the overloaded instruction."""
    for blk in nc.m.functions[0].blocks:
        insts = blk.instructions
        i = 0
        while i < len(insts):
            inst = insts[i]
            si = inst.sync_info
            cap = 2 if isinstance(inst, mybir.InstEventSemaphore) else 1
            if si is not None and si.on_wait and len(si.on_wait) > cap:
                waits = list(si.on_wait)
                si.on_wait = waits[:cap]
                carriers = []
                for w in waits[cap:]:
                    n = mybir.InstNoOp(
                        name=nc.get_next_instruction_name(), ins=[], outs=[]
                    )
                    n.engine = inst.engine
                    n.sync_info = mybir.SyncInfo(on_wait=[w], on_update=[])
                    nc.register_instruction(n)
                    carriers.append(n)
                for j, n in enumerate(carriers):
                    insts.insert(i + j, n)
                i += len(carriers)
            i += 1


def make_tri(nc, out, kind):
    """identity / upper-triangular constant, built in-place on Pool."""
    sq = out.shape[0]
    nc.gpsimd.memset(out, 0.0)
    nc.gpsimd.affine_select(
        out=out,
        in_=out,
        compare_op=(
            mybir.AluOpType.not_equal if kind == "ident" else mybir.AluOpType.is_gt
        ),
        fill=1.0,
        base=0,
        pattern=[[-1, sq]],
        channel_multiplier=1,
    )


def attention_body(tc, consts, pools, q, k, v, out, phase=4):
    """Emit one iteration (per-core shapes). Pools persist across iterations
    so back-to-back iterations pipeline."""
    nc = tc.nc
    identh = consts["identh"]
    umask = consts["umask"]
    mask2 = consts["mask2"]
    w_list = consts["w_list"]
    mask_f = consts["mask_f"]

    (nat_pool, xt_pool, pj_pool, vx_pool, ex_pool, on_pool,
     ps_tr, ps_pj, ps_sc, ps_out) = pools

    # --- all input loads up front (Pool SWDGE cast fp32->fp16) -------------
    srcs = {"k": k, "v": v, "q": q}
    natg = {}
    for g in range(NG):
        for name in ("k", "v", "q"):
            x = srcs[name]
            ng = nat_pool.tile([P, GG, C], FP16, tag="natg", bufs=3 * NG, name="ng")
            nc.gpsimd.dma_start(
                out=ng[:],
                in_=x[g * GG * P : (g + 1) * GG * P, :].rearrange(
                    "(tt p) c -> p tt c", p=P
                ),
            )
            natg[(name, g)] = ng
            if g > 0:
                continue
            widx = {"q": 0, "k": 1, "v": 2}[name]
            if w_list[widx][1] is None:
                w_dram = w_list[widx][0]
                # [c-part, cb, h] fp32 on SP HWDGE (no Pool time), ACT -> fp16
                wf = nat_pool.tile(
                    [P, NCB, HS], F32, tag=f"wf_{name}", bufs=1, name="wf"
                )
                nc.sync.dma_start(
                    out=wf[:], in_=w_dram.rearrange("(cb c) h -> c cb h", c=P)
                )
                wh = nat_pool.tile(
                    [P, NCB, HS], FP16, tag=f"wh_{name}", bufs=1, name="wh"
                )
                nc.scalar.copy(out=wh[:], in_=wf[:])
                w_list[widx] = (w_dram, wh)
            if name == "k" and not consts["built_a"]:
                # identity right behind the first load's descriptor gen
                consts["built_a"] = True
                make_tri(nc, identh[:], "ident")
            if name == "q" and not consts["built_b"]:
                # remaining consts behind the q-load's descriptor gen
                consts["built_b"] = True
                make_tri(nc, umask[:], "umask")
                nc.gpsimd.memset(mask2[:], 1.0)
                nc.gpsimd.memset(mask2[:, 1, 0:P], 0.0)
                nc.vector.tensor_copy(out=mask2[:, 0, 0:P], in_=umask[:])
                nc.vector.tensor_copy(out=mask2[:, 1, P : 2 * P], in_=umask[:])
                mask_i = consts["mask_i"]
                nc.sync.dma_start(
                    out=mask_i[:],
                    in_=consts["mask_dram"].rearrange("(tb p) -> p tb", p=P),
                )
                nc.vector.tensor_copy(out=mask_f[:], in_=mask_i[:])

    if phase < 2:
        dummy = on_pool.tile([P, HS], F32, tag="osb_d")
        nc.vector.tensor_copy(out=dummy[:], in_=natg[("q", NG - 1)][:, 0, 0:HS])
        nc.sync.dma_start(out=out[0:P, :], in_=dummy[:])
        return

    qts, kts, vxs = [], [], []
    evac = [0]

    def frontend_units(g):
        """Emission closures for group g's transposes + projections, so
        attention(g-1) can interleave them into the PE's exp-wait gaps."""
        units = []

        def mk_transpose(name, cbp, holder):
            def u():
                if cbp == 0:
                    holder["xt"] = xt_pool.tile(
                        [P, NCB, TI], FP16, tag="xt", name="xt"
                    )
                xt = holder["xt"]
                tp = ps_tr.tile([P, 2, GG, P], FP16, tag="tr")
                for ci in range(2):
                    cb = 2 * cbp + ci
                    for tt in range(GG):
                        nc.tensor.transpose(
                            tp[:, ci, tt, :],
                            natg[(name, g)][:, tt, cb * P : (cb + 1) * P],
                            identh[:],
                        )
                dst = xt[:, 2 * cbp : 2 * cbp + 2, :].rearrange(
                    "p c (tt ti) -> p c tt ti", tt=GG
                )
                if EVAC_MOD and evac[0] % EVAC_MOD == EVAC_MOD - 1:
                    nc.scalar.copy(out=dst, in_=tp[:])
                else:
                    nc.vector.tensor_copy(out=dst, in_=tp[:])
                evac[0] += 1
            return u

        def mk_proj(name, holder):
            def u():
                wh = w_list[{"q": 0, "k": 1, "v": 2}[name]][1]
                xt = holder["xt"]
                pj = ps_pj.tile([HS, TI], F32, tag="pj")
                for cb in range(NCB):
                    nc.tensor.matmul(
                        pj[:],
                        lhsT=wh[:, cb, :],
                        rhs=xt[:, cb, :],
                        start=(cb == 0),
                        stop=(cb == NCB - 1),
                    )
                if name in ("q", "k"):
                    pt = pj_pool.tile([HS, TI], FP16, tag=f"{name}t", bufs=2 * NG)
                    nc.vector.tensor_copy(out=pt[:], in_=pj[:])
                    (qts if name == "q" else kts).append(pt)
                else:
                    pass
            return u

        def mk_vproj(holder):
            def u():
                # V projected directly into natural [j-part, h]: the xT
                # chunks are the stationary operand and W streams, so no
                # transpose-back is needed
                wh = w_list[2][1]
                xt = holder["xt"]
                vxg = vx_pool.tile([P, GG, HS + 4], FP16, tag="vx", bufs=2 * NG)
                vp = ps_pj.tile([P, GG, HS + 2], F32, tag="pj", name="vp")
                for tt in range(GG):
                    for cb in range(NCB):
                        nc.tensor.matmul(
                            vp[:, tt, 0:HS],
                            lhsT=xt[:, cb, tt * P : (tt + 1) * P],
                            rhs=wh[:, cb, :],
                            start=(cb == 0),
                            stop=(cb == NCB - 1),
                        )
                for tt in range(GG):
                    jt = g * GG + tt
                    nc.vector.tensor_scalar_mul(
                        out=vxg[:, tt, 0:HS],
                        in0=vp[:, tt, 0:HS],
                        scalar1=mask_f[:, jt : jt + 1],
                    )
                nc.vector.tensor_copy(
                    out=vxg[:, :, HS], in_=mask_f[:, g * GG : (g + 1) * GG]
                )
                vxs.append(vxg)
            return u

        for name in ("k", "v", "q"):
            holder = {}
            for cbp in range(NCB // 2):
                units.append(mk_transpose(name, cbp, holder))
            if phase >= 3:
                units.append(
                    mk_vproj(holder) if name == "v" else mk_proj(name, holder)
                )
        return units

    def attention(g, units):
        """Attention i-chunk ic == g: paired fp16 scores/exp one pair ahead
        of A@V, with next-group frontend units drained into exp-wait gaps."""
        njb = 4 * g + 4
        out_ps = ps_out.tile([HS + 1, TI], F32, tag="out")
        exs = {}

        def drain(n):
            for _ in range(n):
                if units:
                    units.pop(0)()

        def pair_scores(pr):
            """Both j-blocks of pair pr -> fp32 PSUM, one paired exp -> fp16.
            Diagonal pairs compute the full [o0:) range for both sub-blocks
            and apply one combined [umask|ones ; 0|umask] multiplicative
            mask over the 256 columns at o0."""
            jb0 = 2 * pr
            o0 = max(0, jb0 * P - g * TI)
            sc = ps_sc.tile([P, 2, TI], F32, tag="sc")
            for i in (0, 1):
                gj, tj = (jb0 + i) // 4, (jb0 + i) % 4
                nc.tensor.matmul(
                    sc[:, i, o0:],
                    lhsT=kts[gj][:, tj * P : (tj + 1) * P],
                    rhs=qts[g][:, o0:],
                    start=True,
                    stop=True,
                )
            ex = ex_pool.tile([P, 2, TI], FP16, tag="ex")
            nc.scalar.activation(
                out=ex[:, :, o0:],
                in_=sc[:, :, o0:],
                func=EXP,
                scale=float(HS) ** -0.5,
            )
            if jb0 >= 4 * g:
                nc.vector.tensor_mul(
                    ex[:, :, o0 : o0 + 2 * P], ex[:, :, o0 : o0 + 2 * P], mask2[:]
                )
            exs[pr] = (ex, o0)

        def av(pr, npr):
            ex, o0 = exs.pop(pr)
            for i in (0, 1):
                jb = 2 * pr + i
                o = max(0, jb * P - g * TI)
                gj, tj = jb // 4, jb % 4
                nc.tensor.matmul(
                    out_ps[:, o:],
                    lhsT=vxs[gj][:, tj, 0 : HS + 1],
                    rhs=ex[:, i, o:],
                    start=(jb == 0),
                    stop=(jb == njb - 1),
                )

        npr = njb // 2
        pair_scores(0)
        for pr in range(1, npr):
            pair_scores(pr)
            drain(DRAIN_N)
            av(pr - 1, npr)
        av(npr - 1, npr)

        # normalize + emit
        oun = on_pool.tile([HS + 1, TI], FP16, tag="oun")
        nc.scalar.copy(out=oun[:], in_=out_ps[:])
        osb = on_pool.tile([P, GG, HS], F32, tag="osb")
        op = ps_out.tile([P, GG, HS + 2], FP16, tag="out", name="op")
        for tt in range(GG):
            nc.tensor.transpose(
                op[:, tt, 0 : HS + 1],
                oun[:, tt * P : (tt + 1) * P],
                identh[0 : HS + 1, 0 : HS + 1],
            )
        rden = on_pool.tile([P, GG], F32, tag="rden")
        nc.vector.reciprocal(out=rden[:], in_=op[:, :, HS])
        for tt in range(GG):
            if tt % MUL_SPLIT == 0:
                nc.vector.tensor_scalar_mul(
                    out=osb[:, tt, :], in0=op[:, tt, 0:HS],
                    scalar1=rden[:, tt : tt + 1],
                )
            else:
                nc.scalar.mul(
                    out=osb[:, tt, :], in_=op[:, tt, 0:HS],
                    mul=rden[:, tt : tt + 1],
                )
        nc.sync.dma_start(
            out=out[g * TI : (g + 1) * TI, :].rearrange("(tt p) h -> p tt h", p=P),
            in_=osb[:],
        )

    # --- pipeline: next-group frontend interleaved into attention ----------
    for u in frontend_units(0):
        u()
    for g in range(NG):
        units = frontend_units(g + 1) if g + 1 < NG else []
        if phase < 4:
            for u in units:
                u()
            if phase >= 3 and g == NG - 1:
                dummy = on_pool.tile([P, HS], F32, tag="osb_d")
                nc.vector.tensor_copy(out=dummy[:], in_=vxs[g][:, 0, 0:HS])
                nc.sync.dma_start(out=out[0:P, :], in_=dummy[:])
            continue
        attention(g, units)
        for u in units:
            u()


def build_nc(n_iters: int = 1, phase: int = 4):
    nc = bass.Bass(trn_type="TRN2", num_devices=B)
    q = nc.declare_dram_parameter("q_vec", [T, C], F32, isOutput=False)
    k = nc.declare_dram_parameter("k_vec", [T, C], F32, isOutput=False)
    v = nc.declare_dram_parameter("v_vec", [T, C], F32, isOutput=False)
    mask = nc.declare_dram_parameter("mask", [T], I32, isOutput=False)
    wq = nc.declare_dram_parameter("Wq", [C, HS], F32, isOutput=False)
    wk = nc.declare_dram_parameter("Wk", [C, HS], F32, isOutput=False)
    wv = nc.declare_dram_parameter("Wv", [C, HS], F32, isOutput=False)
    out = nc.declare_dram_parameter("out", [T, HS], F32, isOutput=True)

    with tile.TileContext(nc) as tc:
        with tc.tile_pool(name="singles", bufs=1) as singles, \
             tc.tile_pool(name="nat", bufs=1) as nat_pool, \
             tc.tile_pool(name="xt", bufs=XT_BUFS) as xt_pool, \
             tc.tile_pool(name="pj", bufs=2) as pj_pool, \
             tc.tile_pool(name="vx", bufs=2) as vx_pool, \
             tc.tile_pool(name="ex", bufs=EX_BUFS) as ex_pool, \
             tc.tile_pool(name="on", bufs=2) as on_pool, \
             tc.tile_pool(name="ps_tr", bufs=2, space="PSUM") as ps_tr, \
             tc.tile_pool(name="ps_pj", bufs=1, space="PSUM") as ps_pj, \
             tc.tile_pool(name="ps_sc", bufs=2, space="PSUM") as ps_sc, \
             tc.tile_pool(name="ps_out", bufs=1, space="PSUM") as ps_out:
            # all constants are emitted lazily inside the first body,
            # interleaved with the startup-critical group-0 loads
            consts = {
                "built_a": False,
                "built_b": False,
                "identh": singles.tile([P, P], FP16, name="identh"),
                "umask": singles.tile([P, P], FP16, name="umask"),
                "mask2": singles.tile([P, 2, 2 * P], FP16, name="mask2"),
                "mask_i": singles.tile([P, NT], I32, name="mask_i"),
                "mask_f": singles.tile([P, NT], F32, name="mask_f"),
                "mask_dram": mask.ap(),
                "w_list": [(wq.ap(), None), (wk.ap(), None), (wv.ap(), None)],
            }
            pools = (nat_pool, xt_pool, pj_pool, vx_pool, ex_pool, on_pool,
                     ps_tr, ps_pj, ps_sc, ps_out)
            for _ in range(n_iters):
                attention_body(
                    tc, consts, pools, q.ap(), k.ap(), v.ap(), out.ap(), phase=phase
                )

    split_excess_waits(nc)
    return nc


# ---------------------------------------------------------------------------
# SPMD runner (compile once, execute via PJRT on the 8 axon cores)
# ---------------------------------------------------------------------------
class _Runner:
    def __init__(self, nc, n_cores=B):
        import jax
        from jax.sharding import Mesh, PartitionSpec
        from jax.experimental.shard_map import shard_map
        from concourse.bass2jax import (
            _bass_exec_p,
            install_neuronx_cc_hook,
            partition_id_tensor,
        )

        install_neuronx_cc_hook()
        self.jax = jax
        self.n_cores = n_cores
        partition_name = (
            nc.partition_id_tensor.name if nc.partition_id_tensor else None
        )

        in_names, out_names, out_avals, zero_outs = [], [], [], []
        for alloc in nc.m.functions[0].allocations:
            if not isinstance(alloc, mybir.MemoryLocationSet):
                continue
            name = alloc.memorylocations[0].name
            if alloc.kind == "ExternalInput":
                if name != partition_name:
                    in_names.append(name)
            elif alloc.kind == "ExternalOutput":
                out_names.append(name)
                shape = tuple(alloc.tensor_shape)
                dtype = mybir.dt.np(alloc.dtype)
                out_avals.append(jax.core.ShapedArray(shape, dtype))
                zero_outs.append(np.zeros(shape, dtype))
        self.in_names = list(in_names)
        self.out_names = out_names
        self.out_avals = out_avals
        self.zero_outs = zero_outs
        n_params = len(in_names)
        self.n_params = n_params

        all_in_names = list(in_names) + list(out_names)
        if partition_name is not None:
            all_in_names.append(partition_name)

        def _body(*args):
            operands = list(args)
            if partition_name is not None:
                operands.append(partition_id_tensor())
            outs = _bass_exec_p.bind(
                *operands,
                out_avals=tuple(out_avals),
                in_names=tuple(all_in_names),
                out_names=tuple(out_names),
                lowering_input_output_aliases=(),
                sim_require_finite=True,
                sim_require_nnan=True,
                nc=nc,
            )
            return tuple(outs)

        devices = jax.devices()[:n_cores]
        mesh = Mesh(np.asarray(devices), ("core",))
        n_outs = len(out_names)
        self.fn = jax.jit(
            shard_map(
                _body,
                mesh=mesh,
                in_specs=(PartitionSpec("core"),) * (n_params + n_outs),
                out_specs=(PartitionSpec("core"),) * n_outs,
                check_rep=False,
            ),
            keep_unused=True,
        )

    def prepare(self, in_maps):
        n = self.n_cores
        per_core = [[np.asarray(m[nm]) for nm in self.in_names] for m in in_maps]
        concat_in = [
            np.concatenate([per_core[c][i] for c in range(n)], axis=0)
            for i in range(self.n_params)
        ]
        concat_zeros = [
            np.zeros((n * z.shape[0], *z.shape[1:]), z.dtype) for z in self.zero_outs
        ]
        self.args = [self.jax.device_put(a) for a in concat_in + concat_zeros]
        return self

    def run(self):
        outs = self.fn(*self.args)
        self.jax.block_until_ready(outs)
        return outs

    def results(self, outs):
        n = self.n_cores
        return [
            {
                nm: np.asarray(outs[i]).reshape(n, *self.out_avals[i].shape)[c]
                for i, nm in enumerate(self.out_names)
            }
            for c in range(n)
        ]


_CACHED = {}


def _get_runner(n_iters: int = 1, phase: int = 4):
    key = (n_iters, phase)
    if key not in _CACHED:
        _CACHED[key] = _Runner(build_nc(n_iters, phase))
    return _CACHED[key]


def kernel(q_vec, k_vec, v_vec, mask, Wq, Wk, Wv):
    q_vec = np.ascontiguousarray(np.asarray(q_vec, dtype=np.float32))
    k_vec = np.ascontiguousarray(np.asarray(k_vec, dtype=np.float32))
    v_vec = np.ascontiguousarray(np.asarray(v_vec, dtype=np.float32))
    mask = np.ascontiguousarray(np.asarray(mask, dtype=np.int32))
    Wq = np.ascontiguousarray(np.asarray(Wq, dtype=np.float32))
    Wk = np.ascontiguousarray(np.asarray(Wk, dtype=np.float32))
    Wv = np.ascontiguousarray(np.asarray(Wv, dtype=np.float32))

    r = _get_runner()
    in_maps = [
        {
            "q_vec": q_vec[b],
            "k_vec": k_vec[b],
            "v_vec": v_vec[b],
            "mask": mask[b],
            "Wq": Wq,
            "Wk": Wk,
            "Wv": Wv,
        }
        for b in range(B)
    ]
    r.prepare(in_maps)
    res = r.results(r.run())
    return np.stack([res[b]["out"] for b in range(B)], axis=0)
